# revision 12
# baseline (speedup 1.0000x reference)
"""ChebConv (K=3) kernel for Trainium2, data-parallel over batch across 8 NeuronCores.

Math (per batch b):
    d       = adj.sum(axis=1)                  (row sums)
    dinv    = (d + 1e-6) ** -0.5,  dsq = (d + 1e-6) ** 0.5
    M       = Dinv @ A @ Dinv      (so L = I - M)
    Tx0 = x, Tx1 = x - M x, Tx2 = 2(Tx1 - M Tx1) - x
    out     = relu(sum_k Txk @ W[k] + sum_k b[k])

Kernel-side reformulation (all scaled tensors, avoids materializing L):
    yk := Dinv @ Txk   (bf16 "weights" for the PE passes)
    y0 = Dinv x
    z1 = M x          (PE pass 1: lhsT = y0 tiles, rhs = scaled-transposed A)
    y1 = y0 - Dinv z1
    z2 = M Tx1        (PE pass 2: lhsT = y1 tiles)
    y2 = 2 y1 - y0 - 2 Dinv z2
    out = relu(Dsq @ (sum_k yk @ W[k]) + bsum)

A is cast to bf16 during the DMA load (SWDGE cast). The transpose of A that the
PE needs (contraction runs over the partition dim) is produced with per-tile
matmuls against diag(dinv) as the moving operand, which simultaneously applies
the output-side Dinv scaling:  ats2[j, i] = A[i, j] * dinv[i].
"""

import numpy as np

B, N, F, K = 8, 2048, 128, 3
P = 128
NT = N // P  # 16
EPS = 1e-6
NCORES = 8

_cache = {}


def _build_nc():
    from contextlib import ExitStack

    import concourse.bacc as bacc
    import concourse.tile as tile
    from concourse import mybir

    f32 = mybir.dt.float32
    bf16 = mybir.dt.bfloat16
    AF = mybir.ActivationFunctionType
    OP = mybir.AluOpType

    nc = bacc.Bacc("TRN2", target_bir_lowering=False, debug=False, num_devices=NCORES)
    adj = nc.dram_tensor("adj", [N, N], f32, kind="ExternalInput").ap()
    x = nc.dram_tensor("x", [N, F], f32, kind="ExternalInput").ap()
    W = nc.dram_tensor("W", [K, F, F], f32, kind="ExternalInput").ap()
    bsum_d = nc.dram_tensor("bsum", [P, F], f32, kind="ExternalInput").ap()
    ident = nc.dram_tensor("ident", [P, P], f32, kind="ExternalInput").ap()
    out = nc.dram_tensor("out", [N, F], f32, kind="ExternalOutput").ap()
    out_t = out.rearrange("(t p) f -> p t f", p=P)

    with ExitStack() as ctx:
        tc = ctx.enter_context(tile.TileContext(nc))
        consts = ctx.enter_context(tc.tile_pool(name="consts", bufs=1))
        apool = ctx.enter_context(tc.tile_pool(name="apool", bufs=4))
        big = ctx.enter_context(tc.tile_pool(name="big", bufs=1))
        small = ctx.enter_context(tc.tile_pool(name="small", bufs=3))
        scr = ctx.enter_context(tc.tile_pool(name="scr", bufs=2))
        ps_acc = ctx.enter_context(tc.tile_pool(name="ps_acc", bufs=1, space="PSUM"))
        ps_t = ctx.enter_context(tc.tile_pool(name="ps_t", bufs=3, space="PSUM"))

        # ---- constants -------------------------------------------------
        ident_bf = consts.tile([P, P], bf16)
        nc.gpsimd.dma_start(out=ident_bf, in_=ident)
        w_bf = consts.tile([P, K, F], bf16)
        nc.gpsimd.dma_start(out=w_bf, in_=W.rearrange("k i o -> i k o"))
        bsum = consts.tile([P, F], f32)
        nc.sync.dma_start(out=bsum, in_=bsum_d)
        eps_sb = consts.tile([P, 1], f32)
        nc.vector.memset(eps_sb, EPS)

        # per-node scalars, [P, NT]: column r holds values for node tile r
        dsq = consts.tile([P, NT], f32)
        dinv = consts.tile([P, NT], f32)
        ndinv = consts.tile([P, NT], f32)
        n2dinv = consts.tile([P, NT], f32)

        y0 = big.tile([P, NT, F], bf16)
        y1 = big.tile([P, NT, F], bf16)
        y2 = big.tile([P, NT, F], bf16)
        tt = big.tile([P, NT, F], bf16)
        ats2 = big.tile([P, NT, N], bf16)  # [j_in_tile, c(j tile), i]: A[i,j]*dinv[i]
        yT = big.tile([P, K, N], bf16)     # transposed yk: [f, k, i]

        # ---- streaming phase: load A tiles, reduce d, transpose+scale --
        for r in range(NT):
            a_t = apool.tile([P, N], bf16, tag="a")
            nc.gpsimd.dma_start(out=a_t, in_=adj[r * P:(r + 1) * P, :])
            x_t = apool.tile([P, F], f32, tag="x")
            nc.sync.dma_start(out=x_t, in_=x[r * P:(r + 1) * P, :])

            d_r = small.tile([P, 1], f32, tag="d")
            s = scr.tile([P, N], bf16, tag="s")
            if r % 2 == 0:
                nc.vector.tensor_scalar(
                    out=s, in0=a_t, scalar1=1.0, scalar2=0.0, op0=OP.mult,
                    op1=OP.add, accum_out=d_r)
            else:
                nc.scalar.activation(out=s, in_=a_t, func=AF.Identity, accum_out=d_r)

            nc.scalar.activation(out=dsq[:, r:r + 1], in_=d_r, func=AF.Sqrt,
                                 bias=eps_sb)
            nc.vector.reciprocal(out=dinv[:, r:r + 1], in_=dsq[:, r:r + 1])
            nc.vector.tensor_scalar(out=ndinv[:, r:r + 1], in0=dinv[:, r:r + 1],
                                    scalar1=-1.0, scalar2=None, op0=OP.mult)
            nc.vector.tensor_scalar(out=n2dinv[:, r:r + 1], in0=dinv[:, r:r + 1],
                                    scalar1=-2.0, scalar2=None, op0=OP.mult)
            diag_r = small.tile([P, P], bf16, tag="diag")
            nc.vector.tensor_scalar(out=diag_r, in0=ident_bf,
                                    scalar1=dinv[:, r:r + 1], scalar2=None,
                                    op0=OP.mult)
            nc.vector.tensor_scalar(out=y0[:, r, :], in0=x_t,
                                    scalar1=dinv[:, r:r + 1], scalar2=None,
                                    op0=OP.mult)
            # transpose + dinv[i]-scale A tile row r: 16 (128x128) matmuls
            for g in range(4):
                pt = ps_t.tile([P, 4, P], f32, tag="t")
                for q in range(4):
                    c = 4 * g + q
                    nc.tensor.matmul(pt[:, q, :], lhsT=a_t[:, c * P:(c + 1) * P],
                                     rhs=diag_r, start=True, stop=True)
                nc.any.tensor_copy(out=ats2[:, 4 * g:4 * g + 4, r * P:(r + 1) * P],
                                   in_=pt)

        # ---- chebyshev PE passes --------------------------------------
        def cheb_pass(weights, zacc):
            for c in range(NT):
                for nch in range(4):
                    nc.tensor.matmul(zacc[:, nch * 512:(nch + 1) * 512],
                                     lhsT=weights[:, c, :],
                                     rhs=ats2[:, c, nch * 512:(nch + 1) * 512],
                                     start=(c == 0), stop=(c == NT - 1))

        def z_to_nat_and_combine(zacc, zbf, scal, base, ydst):
            # zbf = bf16 cast of z (transposed layout); then per node tile:
            # ydst[r] = znat[r] * scal[r] + base[r]
            for nch in range(4):
                nc.any.tensor_copy(out=zbf[:, nch * 512:(nch + 1) * 512],
                                   in_=zacc[:, nch * 512:(nch + 1) * 512])
            for g in range(4):
                zn = ps_t.tile([P, 4, P], f32, tag="t")
                for q in range(4):
                    r = 4 * g + q
                    nc.tensor.matmul(zn[:, q, :], lhsT=zbf[:, r * P:(r + 1) * P],
                                     rhs=ident_bf, start=True, stop=True)
                for q in range(4):
                    r = 4 * g + q
                    nc.vector.scalar_tensor_tensor(
                        out=ydst[:, r, :], in0=zn[:, q, :], scalar=scal[:, r:r + 1],
                        in1=base[:, r, :], op0=OP.mult, op1=OP.add)

        z1 = ps_acc.tile([P, N], f32, tag="acc")
        cheb_pass(y0, z1)
        z1bf = big.tile([P, N], bf16, tag="zbf")
        z_to_nat_and_combine(z1, z1bf, ndinv, y0, y1)   # y1 = y0 - dinv*z1

        z2 = ps_acc.tile([P, N], f32, tag="acc")
        cheb_pass(y1, z2)
        for r in range(NT):  # tt = 2*y1 - y0
            nc.vector.scalar_tensor_tensor(
                out=tt[:, r, :], in0=y1[:, r, :], scalar=2.0, in1=y0[:, r, :],
                op0=OP.mult, op1=OP.subtract)
        z2bf = big.tile([P, N], bf16, tag="zbf2")
        z_to_nat_and_combine(z2, z2bf, n2dinv, tt, y2)  # y2 = tt - 2*dinv*z2

        # ---- transpose y0/y1/y2 for the output matmuls ----------------
        for k3, ysrc in enumerate((y0, y1, y2)):
            for g in range(4):
                pt = ps_t.tile([P, 4, P], f32, tag="t")
                for q in range(4):
                    r = 4 * g + q
                    nc.tensor.matmul(pt[:, q, :], lhsT=ysrc[:, r, :], rhs=ident_bf,
                                     start=True, stop=True)
                nc.any.tensor_copy(out=yT[:, k3, g * 512:(g + 1) * 512], in_=pt)

        # ---- output layer: out = relu(dsq * (sum_k yk @ Wk) + bsum) ---
        for g in range(4):
            og = small.tile([P, 4, F], f32, tag="og")
            for q in range(4):
                r = 4 * g + q
                oc = ps_t.tile([P, F], f32, tag="t")
                for k3 in range(K):
                    nc.tensor.matmul(oc, lhsT=yT[:, k3, r * P:(r + 1) * P],
                                     rhs=w_bf[:, k3, :],
                                     start=(k3 == 0), stop=(k3 == K - 1))
                tmp = small.tile([P, F], f32, tag="tmp")
                nc.vector.scalar_tensor_tensor(
                    out=tmp, in0=oc, scalar=dsq[:, r:r + 1], in1=bsum,
                    op0=OP.mult, op1=OP.add)
                nc.scalar.activation(out=og[:, q, :], in_=tmp, func=AF.Relu)
            nc.sync.dma_start(out=out_t[:, 4 * g:4 * g + 4, :], in_=og)

    nc.compile()
    return nc


def _get_nc():
    if "nc" not in _cache:
        _cache["nc"] = _build_nc()
    return _cache["nc"]


def make_in_maps(x, adj, W, b):
    ident = np.eye(P, dtype=np.float32)
    x = np.ascontiguousarray(np.asarray(x, dtype=np.float32))
    adj = np.ascontiguousarray(np.asarray(adj, dtype=np.float32))
    Wf = np.ascontiguousarray(np.asarray(W, dtype=np.float32))
    bf = np.asarray(b, dtype=np.float32)
    bsum = np.ascontiguousarray(
        np.broadcast_to(bf.sum(axis=0), (P, F)).astype(np.float32))
    return [
        {"adj": adj[c], "x": x[c], "W": Wf, "bsum": bsum, "ident": ident}
        for c in range(NCORES)
    ]


def run_raw(x, adj, W, b, **kwargs):
    from concourse import bass_utils

    nc = _get_nc()
    in_maps = make_in_maps(x, adj, W, b)
    res = bass_utils.run_bass_kernel_spmd(nc, in_maps,
                                          core_ids=list(range(NCORES)), **kwargs)
    out = np.stack([res.results[c]["out"] for c in range(NCORES)], axis=0)
    return out.astype(np.float32), res


def kernel(x, adj, W, b):
    out, _ = run_raw(x, adj, W, b)
    return out


# revision 13
# speedup vs baseline: 1.0241x; 1.0241x over previous
"""ChebConv (K=3) kernel for Trainium2, data-parallel over batch across 8 NeuronCores.

Math (per batch b):
    d       = adj.sum(axis=1)                  (row sums)
    dinv    = (d + 1e-6) ** -0.5,  dsq = (d + 1e-6) ** 0.5
    M       = Dinv @ A @ Dinv      (so L = I - M)
    Tx0 = x, Tx1 = x - M x, Tx2 = 2(Tx1 - M Tx1) - x
    out     = relu(sum_k Txk @ W[k] + sum_k b[k])

Kernel-side reformulation (all scaled tensors, avoids materializing L):
    yk := Dinv @ Txk   (bf16 "weights" for the PE passes)
    y0 = Dinv x
    z1 = M x          (PE pass 1: lhsT = y0 tiles, rhs = scaled-transposed A)
    y1 = y0 - Dinv z1
    z2 = M Tx1        (PE pass 2: lhsT = y1 tiles)
    y2 = 2 y1 - y0 - 2 Dinv z2
    out = relu(Dsq @ (sum_k yk @ W[k]) + bsum)

A row-tiles stream in as fp32 over HWDGE; a single fused DVE/ACT op per tile
does the bf16 cast AND the row-sum reduction (accum_out). The transpose of A
that the PE needs is produced by per-tile matmuls against diag(dinv) as the
moving operand, which also applies the output-side Dinv scale:
ats2[j, i] = A[i, j] * dinv[i].  Pass 1 is emitted triangularly inside the
load loop so it overlaps the DMA stream.
"""

import numpy as np

B, N, F, K = 8, 2048, 128, 3
P = 128
NT = N // P  # 16
EPS = 1e-6
NCORES = 8

_cache = {}


def _build_nc():
    from contextlib import ExitStack

    import concourse.bacc as bacc
    import concourse.tile as tile
    from concourse import mybir

    f32 = mybir.dt.float32
    bf16 = mybir.dt.bfloat16
    AF = mybir.ActivationFunctionType
    OP = mybir.AluOpType

    nc = bacc.Bacc("TRN2", target_bir_lowering=False, debug=False, num_devices=NCORES)
    adj = nc.dram_tensor("adj", [N, N], f32, kind="ExternalInput").ap()
    x = nc.dram_tensor("x", [N, F], f32, kind="ExternalInput").ap()
    W = nc.dram_tensor("W", [K, F, F], f32, kind="ExternalInput").ap()
    bsum_d = nc.dram_tensor("bsum", [P, F], f32, kind="ExternalInput").ap()
    ident = nc.dram_tensor("ident", [P, P], f32, kind="ExternalInput").ap()
    out = nc.dram_tensor("out", [N, F], f32, kind="ExternalOutput").ap()
    out_t = out.rearrange("(t p) f -> p t f", p=P)

    with ExitStack() as ctx:
        tc = ctx.enter_context(tile.TileContext(nc))
        consts = ctx.enter_context(tc.tile_pool(name="consts", bufs=1))
        afp = ctx.enter_context(tc.tile_pool(name="afp", bufs=3))
        abp = ctx.enter_context(tc.tile_pool(name="abp", bufs=4))
        big = ctx.enter_context(tc.tile_pool(name="big", bufs=1))
        small = ctx.enter_context(tc.tile_pool(name="small", bufs=3))
        ps_acc = ctx.enter_context(tc.tile_pool(name="ps_acc", bufs=1, space="PSUM"))
        ps_t = ctx.enter_context(tc.tile_pool(name="ps_t", bufs=4, space="PSUM"))

        # ---- constants -------------------------------------------------
        ident_bf = consts.tile([P, P], bf16)
        nc.gpsimd.dma_start(out=ident_bf, in_=ident)
        w_bf = consts.tile([P, K, F], bf16)
        nc.gpsimd.dma_start(out=w_bf, in_=W.rearrange("k i o -> i k o"))
        bsum = consts.tile([P, F], f32)
        nc.sync.dma_start(out=bsum, in_=bsum_d)
        eps_sb = consts.tile([P, 1], f32)
        nc.vector.memset(eps_sb, EPS)

        # per-node scalars, [P, NT]: column r holds values for node tile r
        dsq = consts.tile([P, NT], f32)
        dinv = consts.tile([P, NT], f32)
        ndinv = consts.tile([P, NT], f32)
        n2dinv = consts.tile([P, NT], f32)

        y0 = big.tile([P, NT, F], bf16)
        y1 = big.tile([P, NT, F], bf16)
        y2 = big.tile([P, NT, F], bf16)
        tt = big.tile([P, NT, F], bf16)
        ats2 = big.tile([P, NT, N], bf16)  # [j_in_tile, c(j tile), i]: A[i,j]*dinv[i]
        yT = big.tile([P, K, N], bf16)     # transposed yk: [f, k, i]

        z1 = ps_acc.tile([P, N], f32, tag="acc")

        # ---- streaming phase: load A, fused cast+reduce, transpose,
        #      triangular pass-1 (overlaps the DMA stream) ----------------
        pt_y0 = None
        for r in range(NT):
            a_f = afp.tile([P, N], f32, tag="af")
            nc.sync.dma_start(out=a_f, in_=adj[r * P:(r + 1) * P, :])
            x_t = afp.tile([P, F], f32, tag="x")
            nc.sync.dma_start(out=x_t, in_=x[r * P:(r + 1) * P, :])

            # fused fp32->bf16 cast + row-sum (alternate DVE / ACT)
            a_t = abp.tile([P, N], bf16, tag="a")
            d_r = small.tile([P, 1], f32, tag="d")
            if r % 2 == 0:
                nc.vector.tensor_scalar(
                    out=a_t, in0=a_f, scalar1=1.0, scalar2=0.0, op0=OP.mult,
                    op1=OP.add, accum_out=d_r)
            else:
                nc.scalar.activation(out=a_t, in_=a_f, func=AF.Identity,
                                     accum_out=d_r)

            nc.scalar.activation(out=dsq[:, r:r + 1], in_=d_r, func=AF.Sqrt,
                                 bias=eps_sb)
            nc.vector.reciprocal(out=dinv[:, r:r + 1], in_=dsq[:, r:r + 1])
            nc.vector.tensor_scalar(out=ndinv[:, r:r + 1], in0=dinv[:, r:r + 1],
                                    scalar1=-1.0, scalar2=None, op0=OP.mult)
            nc.vector.tensor_scalar(out=n2dinv[:, r:r + 1], in0=dinv[:, r:r + 1],
                                    scalar1=-2.0, scalar2=None, op0=OP.mult)
            diag_r = small.tile([P, P], bf16, tag="diag")
            nc.vector.tensor_scalar(out=diag_r, in0=ident_bf,
                                    scalar1=dinv[:, r:r + 1], scalar2=None,
                                    op0=OP.mult)
            nc.vector.tensor_scalar(out=y0[:, r, :], in0=x_t,
                                    scalar1=dinv[:, r:r + 1], scalar2=None,
                                    op0=OP.mult)
            # transpose + dinv[i]-scale A tile row r: 16 (128x128) matmuls
            for g in range(4):
                pt = ps_t.tile([P, 4, P], f32, tag="t")
                for q in range(4):
                    c = 4 * g + q
                    nc.tensor.matmul(pt[:, q, :], lhsT=a_t[:, c * P:(c + 1) * P],
                                     rhs=diag_r, start=True, stop=True)
                if g % 2 == 0:
                    nc.vector.tensor_copy(
                        out=ats2[:, 4 * g:4 * g + 4, r * P:(r + 1) * P], in_=pt)
                else:
                    nc.scalar.copy(
                        out=ats2[:, 4 * g:4 * g + 4, r * P:(r + 1) * P], in_=pt)

            # transpose y0 tile r into yT[:, 0, :] (grouped by 4)
            if r % 4 == 0:
                pt_y0 = ps_t.tile([P, 4, P], f32, tag="t")
            nc.tensor.matmul(pt_y0[:, r % 4, :], lhsT=y0[:, r, :], rhs=ident_bf,
                             start=True, stop=True)
            if r % 4 == 3:
                nc.scalar.copy(out=yT[:, 0, (r - 3) * P:(r + 1) * P], in_=pt_y0)

            # triangular pass-1 terms that became ready with tile r:
            # (a) strip r, weight blocks c <= r
            for c in range(r + 1):
                nc.tensor.matmul(z1[:, r * P:(r + 1) * P], lhsT=y0[:, c, :],
                                 rhs=ats2[:, c, r * P:(r + 1) * P],
                                 start=(r % 4 == 0 and c == 0),
                                 stop=(c == NT - 1), skip_group_check=True)
            # (b) older strips s < r with new weight block c = r (bank chunks)
            for sg in range((r + 3) // 4):
                lo = 4 * sg
                hi = min(lo + 4, r)  # strips [lo, hi)
                nc.tensor.matmul(z1[:, lo * P:hi * P], lhsT=y0[:, r, :],
                                 rhs=ats2[:, r, lo * P:hi * P],
                                 start=False, stop=(r == NT - 1),
                                 skip_group_check=True)

        # ---- recurrence + pass 2 --------------------------------------
        def z_to_nat_and_combine(zacc, zbf, scal, base, ydst):
            # zbf = bf16 cast of z (transposed layout); then per node tile:
            # ydst[r] = znat[r] * scal[r] + base[r]
            for nch in range(4):
                nc.any.tensor_copy(out=zbf[:, nch * 512:(nch + 1) * 512],
                                   in_=zacc[:, nch * 512:(nch + 1) * 512])
            for g in range(4):
                zn = ps_t.tile([P, 4, P], f32, tag="t")
                for q in range(4):
                    r = 4 * g + q
                    nc.tensor.matmul(zn[:, q, :], lhsT=zbf[:, r * P:(r + 1) * P],
                                     rhs=ident_bf, start=True, stop=True)
                for q in range(4):
                    r = 4 * g + q
                    nc.vector.scalar_tensor_tensor(
                        out=ydst[:, r, :], in0=zn[:, q, :], scalar=scal[:, r:r + 1],
                        in1=base[:, r, :], op0=OP.mult, op1=OP.add)

        def cheb_pass(weights, zacc):
            for c in range(NT):
                for nch in range(4):
                    nc.tensor.matmul(zacc[:, nch * 512:(nch + 1) * 512],
                                     lhsT=weights[:, c, :],
                                     rhs=ats2[:, c, nch * 512:(nch + 1) * 512],
                                     start=(c == 0), stop=(c == NT - 1))

        z1bf = big.tile([P, N], bf16, tag="zbf")
        z_to_nat_and_combine(z1, z1bf, ndinv, y0, y1)   # y1 = y0 - dinv*z1

        z2 = ps_acc.tile([P, N], f32, tag="acc")
        cheb_pass(y1, z2)

        # transpose y1 into yT[:, 1, :] and compute tt = 2*y1 - y0
        # (these only need y1; they overlap pass 2)
        for g in range(4):
            pt = ps_t.tile([P, 4, P], f32, tag="t")
            for q in range(4):
                r = 4 * g + q
                nc.tensor.matmul(pt[:, q, :], lhsT=y1[:, r, :], rhs=ident_bf,
                                 start=True, stop=True)
            nc.scalar.copy(out=yT[:, 1, g * 512:(g + 1) * 512], in_=pt)
        for r in range(NT):
            nc.vector.scalar_tensor_tensor(
                out=tt[:, r, :], in0=y1[:, r, :], scalar=2.0, in1=y0[:, r, :],
                op0=OP.mult, op1=OP.subtract)

        z2bf = big.tile([P, N], bf16, tag="zbf2")
        z_to_nat_and_combine(z2, z2bf, n2dinv, tt, y2)  # y2 = tt - 2*dinv*z2

        # ---- transpose y2, output layer -------------------------------
        for g in range(4):
            pt = ps_t.tile([P, 4, P], f32, tag="t")
            for q in range(4):
                r = 4 * g + q
                nc.tensor.matmul(pt[:, q, :], lhsT=y2[:, r, :], rhs=ident_bf,
                                 start=True, stop=True)
            nc.scalar.copy(out=yT[:, 2, g * 512:(g + 1) * 512], in_=pt)

        # out = relu(dsq * (sum_k yk @ Wk) + bsum)
        for g in range(4):
            og = small.tile([P, 4, F], f32, tag="og")
            for q in range(4):
                r = 4 * g + q
                oc = ps_t.tile([P, F], f32, tag="t")
                for k3 in range(K):
                    nc.tensor.matmul(oc, lhsT=yT[:, k3, r * P:(r + 1) * P],
                                     rhs=w_bf[:, k3, :],
                                     start=(k3 == 0), stop=(k3 == K - 1))
                tmp = small.tile([P, F], f32, tag="tmp")
                nc.vector.scalar_tensor_tensor(
                    out=tmp, in0=oc, scalar=dsq[:, r:r + 1], in1=bsum,
                    op0=OP.mult, op1=OP.add)
                nc.scalar.activation(out=og[:, q, :], in_=tmp, func=AF.Relu)
            nc.sync.dma_start(out=out_t[:, 4 * g:4 * g + 4, :], in_=og)

    nc.compile()
    return nc


def _get_nc():
    if "nc" not in _cache:
        _cache["nc"] = _build_nc()
    return _cache["nc"]


def make_in_maps(x, adj, W, b):
    ident = np.eye(P, dtype=np.float32)
    x = np.ascontiguousarray(np.asarray(x, dtype=np.float32))
    adj = np.ascontiguousarray(np.asarray(adj, dtype=np.float32))
    Wf = np.ascontiguousarray(np.asarray(W, dtype=np.float32))
    bf = np.asarray(b, dtype=np.float32)
    bsum = np.ascontiguousarray(
        np.broadcast_to(bf.sum(axis=0), (P, F)).astype(np.float32))
    return [
        {"adj": adj[c], "x": x[c], "W": Wf, "bsum": bsum, "ident": ident}
        for c in range(NCORES)
    ]


def run_raw(x, adj, W, b, **kwargs):
    from concourse import bass_utils

    nc = _get_nc()
    in_maps = make_in_maps(x, adj, W, b)
    res = bass_utils.run_bass_kernel_spmd(nc, in_maps,
                                          core_ids=list(range(NCORES)), **kwargs)
    out = np.stack([res.results[c]["out"] for c in range(NCORES)], axis=0)
    return out.astype(np.float32), res


def kernel(x, adj, W, b):
    out, _ = run_raw(x, adj, W, b)
    return out
